# revision 30
# baseline (speedup 1.0000x reference)
"""Trainium2 kernel for nn_EnhancedHybridModel (hybrid MLP + 8-qubit circuit).

Reformulation (exact, up to f32 rounding):
  * BatchNorms are inference-mode -> folded into the adjacent Linear.
  * The quantum circuit after RY-encoding uses shared weights, so it is one
    fixed complex matrix M (256x256).  The encoded state is a REAL product
    vector s[b] = kron_i [cos(pre_i/2), -sin(pre_i/2)].
  * q_out = |M s|^2 @ Z  ->  y = [Re M; Im M] @ s  (512x256 matmul),
    then q_out @ W4eff.T folds with the Z-projection into M4 (512x32):
    h4 = relu(y^2 @ M4 + b4eff).

Data parallel over 8 NeuronCores: batch 65536 -> 8192 rows/core.

v3 strategy (on top of the v2 batch-major pipeline):
  * 6 of 8 [128,128] state transposes run on the PE (transpose-mode writes
    into a PSUM tile laid out as sT[256:1024], one DVE copy evacuates the
    lot); 2 ride the otherwise-idle Sync XBAR-DMA path straight into sT.
    Issued at the START of each iteration (input sB is 2 iterations old)
    so the DMA has a full iteration of slack and the early sT copy frees
    the shared PSUM bank before this iteration's I/J/K matmuls claim it.
  * Small matmuls pack into disjoint PE sub-tiles via base-partition
    placement: W5 at SBUF rows 32:64 (row band 1, output at PSUM 64:80),
    W6 at rows 64:80 (band 2, output at PSUM 0:1).  With W1 in band 0 the
    A/J/K matmuls are issued adjacently and run concurrently in the array.
  * y^2 is squared in [128,1024] pair-ACTs (two PSUM banks per y-pair tile)
    halving the per-op overhead; PSUM stays within 8 banks by ring-sharing
    one pool between the mlb (h4p/h5p/kp) tile and the transpose target.
  * Engine balance: Scalar = tanh/sin/squares/out-copy, Vector = relu+bias
    evacuations + sT copy, GpSimd = the whole kron build chain.
  * PSUM ring reuse is always cross-iteration (phase order A,J,K,C,I,H,B)
    so no PE matmul waits on a same-iteration evacuation.
"""

import numpy as np

import concourse.bass as bass
import concourse.mybir as mybir
import concourse.tile as tile
from concourse import bacc
from concourse.masks import make_identity
from concourse.bass_utils import run_bass_kernel_spmd

F32 = mybir.dt.float32
F16 = mybir.dt.float16
AF = mybir.ActivationFunctionType
ALU = mybir.AluOpType

N_CORES = 8
BATCH = 65536
B_CORE = BATCH // N_CORES  # 8192
COLS = 512  # batch columns per tile (one PSUM bank of f32)
NTILES = B_CORE // COLS  # 16

N_QUBITS = 8
N_LAYERS = 3
DIM = 256
EPS = 1e-5

# ---------------------------------------------------------------- host math

_idx = np.arange(DIM)
_CNOT_PERMS = []
for _i in range(N_QUBITS):
    for _j in range(_i + 1, N_QUBITS):
        _c = (_idx >> (N_QUBITS - 1 - _i)) & 1
        _CNOT_PERMS.append(np.where(_c == 1, _idx ^ (1 << (N_QUBITS - 1 - _j)), _idx))
_Z_SIGNS = np.stack(
    [1.0 - 2.0 * ((_idx >> (N_QUBITS - 1 - i)) & 1) for i in range(N_QUBITS)], axis=1
).astype(np.float64)


def _rx(t):
    c, s = np.cos(t / 2), -1j * np.sin(t / 2)
    return np.array([[c, s], [s, c]], np.complex128)


def _ry(t):
    c, s = np.cos(t / 2), np.sin(t / 2)
    return np.array([[c, -s], [s, c]], np.complex128)


def _rz(t):
    e = np.exp(-0.5j * t)
    return np.array([[e, 0], [0, np.conj(e)]], np.complex128)


def _apply_gate(M, G, w):
    # reference einsum('st,bpsq->bptq', U, state): state'[t] = sum_s U[s,t] state[s]
    left = 2**w
    Mr = M.reshape(left, 2, -1, DIM)
    return np.einsum("st,psqj->ptqj", G, Mr).reshape(DIM, DIM)


def _build_circuit_matrix(q_weights):
    qw = np.asarray(q_weights, np.float64)
    M = np.eye(DIM, dtype=np.complex128)
    for l in range(N_LAYERS):
        for i in range(N_QUBITS):
            M = _apply_gate(M, _rx(qw[l, i, 0]), i)
            M = _apply_gate(M, _ry(qw[l, i, 1]), i)
            M = _apply_gate(M, _rz(qw[l, i, 2]), i)
        for perm in _CNOT_PERMS:
            M = M[perm, :]
    return M


def _fold_bn(W, b, g, bt, m, v):
    sc = np.asarray(g, np.float64) / np.sqrt(np.asarray(v, np.float64) + EPS)
    Weff = sc[:, None] * np.asarray(W, np.float64)
    beff = (np.asarray(b, np.float64) - np.asarray(m, np.float64)) * sc + np.asarray(
        bt, np.float64
    )
    return Weff, beff


WCOLS = 1369


def _prep_consts(inputs):
    f = {k: np.asarray(v, np.float64) for k, v in inputs.items() if k != "x"}
    W1e, b1e = _fold_bn(f["W1"], f["b1"], f["g1"], f["bt1"], f["m1"], f["v1"])
    W2e, b2e = _fold_bn(f["W2"], f["b2"], f["g2"], f["bt2"], f["m2"], f["v2"])
    W4e, b4e = _fold_bn(f["W4"], f["b4"], f["g4"], f["bt4"], f["m4"], f["v4"])
    M = _build_circuit_matrix(f["q_weights"])
    C = np.concatenate([M.real, M.imag], axis=0)  # (512, 256)
    Zst = np.concatenate([_Z_SIGNS, _Z_SIGNS], axis=0)  # (512, 8)
    M4 = Zst @ W4e.T  # (512, 32)

    bf = np.float16
    # WPACK fp16 [128, 1369]: ct | w2 | w1 | w3aug | w5@rows32 | w6@rows64 | m4
    wpk = np.zeros((128, WCOLS), bf)
    CT = np.ascontiguousarray(C.T).astype(bf)  # (256, 512)
    wpk[:, 0:512] = CT[0:128]
    wpk[:, 512:1024] = CT[128:256]
    wpk[0:128, 1024:1088] = np.ascontiguousarray(W2e.T).astype(bf)
    wpk[0:16, 1088:1216] = np.ascontiguousarray(W1e.T).astype(bf)
    wpk[0:64, 1216:1224] = np.ascontiguousarray(f["W3"].T).astype(bf)  # (64, 8)
    wpk[64:65, 1216:1224] = np.asarray(f["b3"]).reshape(1, 8).astype(bf)
    # W5 staged at partitions 32:64 so the J matmul runs in PE row band 1
    wpk[32:64, 1224:1240] = np.ascontiguousarray(f["W5"].T).astype(bf)
    # W6 staged at partitions 64:80 (row band 2)
    wpk[64:80, 1240:1241] = np.ascontiguousarray(f["W6"].T).astype(bf)
    # second-slot copies for the epilog-compressed last tile (tile 15 runs
    # its I/J/K in the same iterations as tile 14, in different PE bands)
    wpk[96:128, 1224:1240] = np.ascontiguousarray(f["W5"].T).astype(bf)
    wpk[32:48, 1240:1241] = np.ascontiguousarray(f["W6"].T).astype(bf)
    M4b = M4.astype(bf)  # (512, 32)
    for c in range(4):
        wpk[:, 1241 + 32 * c : 1241 + 32 * (c + 1)] = M4b[128 * c : 128 * (c + 1)]
    # BIASES f32 [128, 6]: b1 | b2 | b4@rows32 | b5@rows64 | pi/2 | 0
    bs = np.zeros((128, 6), np.float32)
    bs[0:128, 0] = b1e
    bs[0:64, 1] = b2e
    bs[32:64, 2] = b4e
    bs[96:128, 2] = b4e  # second slot (epilog tile 15)
    bs[64:80, 3] = f["b5"]
    bs[32:48, 3] = f["b5"]  # second slot (epilog tile 15)
    bs[0:128, 4] = np.pi / 2
    return {"WPACK": wpk, "BIASES": bs}, float(np.asarray(f["b6"]).reshape(()))


# ------------------------------------------------------------- bass program


def _ap(t, offset, dims):
    """Custom free-dim access pattern on a tile: keep its partition dim."""
    a = t[:]
    return bass.AP(a.tensor, a.offset + offset, [list(a.ap[0])] + [list(d) for d in dims])


def _build_nc():
    nc = bacc.Bacc("TRN2", target_bir_lowering=False, debug=False)

    xt = nc.dram_tensor("xt", [16, B_CORE], F16, kind="ExternalInput")
    wpk_d = nc.dram_tensor("WPACK", [128, WCOLS], F16, kind="ExternalInput")
    bs_d = nc.dram_tensor("BIASES", [128, 6], F32, kind="ExternalInput")
    out_d = nc.dram_tensor("out", [1, B_CORE], F32, kind="ExternalOutput")

    with tile.TileContext(nc) as tc:
        with (
            tc.tile_pool(name="const", bufs=1) as cp,
            tc.tile_pool(name="work", bufs=3) as wp,
            tc.tile_pool(name="pmlp", bufs=2, space="PSUM") as pmlp,
            tc.tile_pool(name="py", bufs=2, space="PSUM") as py,
            tc.tile_pool(name="pb", bufs=2, space="PSUM") as pb,
        ):
            ident = cp.tile([128, 128], F16)
            make_identity(nc, ident[:])
            wpk = cp.tile([128, WCOLS], F16)
            # small weights (w1/w2/w3/w5/w6/m4) first so the MLP phases can
            # start while the big circuit matrix streams in behind them; the
            # big ct block is issued LAST (after x/biases) since the H phase
            # only needs it several iterations in
            nc.scalar.dma_start(wpk[:, 1024:WCOLS], wpk_d[:, 1024:WCOLS])
            bs = cp.tile([128, 6], F32)
            tblpin = cp.tile([1, 1], F16, name="tblpin")
            # pin the ACT table set that Sin/Tanh/Square/Copy live in, so
            # the one table load happens here (overlapped with the DMAs)
            nc.scalar.activation(tblpin[:], bs[0:1, 0:1], AF.Silu)
            ct = wpk[:, 0:1024]
            w2 = wpk[:, 1024:1088]
            w1 = wpk[0:16, 1088:1216]
            w3a = wpk[0:65, 1216:1224]
            w5 = wpk[32:64, 1224:1240]
            w6 = wpk[64:80, 1240:1241]
            w5b = wpk[96:128, 1224:1240]
            w6b = wpk[32:48, 1240:1241]
            m4 = wpk[:, 1241:1369]
            b1 = bs[0:128, 0:1]
            b2 = bs[0:64, 1:2]
            b4 = bs[32:64, 2:3]
            b4b = bs[96:128, 2:3]
            b5 = bs[64:80, 3:4]
            b5b = bs[32:48, 3:4]
            pi2 = bs[0:128, 4:5]
            zero = bs[0:128, 5:6]
            xg = []
            for g in range(4):
                xg.append(cp.tile([16, 4 * COLS], F16, name=f"xg{g}", tag=f"xg{g}"))
                nc.sync.dma_start(xg[g][:], xt[:, 4 * COLS * g : 4 * COLS * (g + 1)])
                if g == 0:
                    nc.sync.dma_start(bs[:], bs_d[:])
            nc.scalar.dma_start(wpk[:, 0:1024], wpk_d[:, 0:1024])

            # row 0 collects tiles 0-14; row 64 collects the epilog-
            # compressed tile 15 (its kp lands at PSUM partition 64)
            out_all = cp.tile([65, B_CORE], F32)
            mm = nc.tensor.matmul

            # prewrite the ones-row into the h2 ring buffers (row 64 is the
            # bias row for the data-stationary W3 matmul; the per-tile act
            # only writes rows 0:64, so the ones persist across slot reuse)
            for _ in range(3):
                h2pre = wp.tile([65, COLS], F16, tag="h2", name="h2pre")
                nc.gpsimd.memset(h2pre[64:65, :], 1.0)

            h1 = [None] * NTILES
            h2 = [None] * NTILES
            pre = [None] * NTILES
            csA = [None] * NTILES
            sB = [None] * NTILES
            sT = [None] * NTILES
            gmap = [None] * NTILES
            sqa = [None] * NTILES
            sqb = [None] * NTILES
            h4 = [None] * NTILES
            h5 = [None] * NTILES
            yps = [None] * NTILES
            mlb_cur = [None]  # one shared [128, COLS] PSUM tile per iteration

            # compressed-prologue schedule: early phases run 2 tiles/iter
            # (PE is H-idle during pipe fill), steady state 1 tile/iter.
            PROLOG = {
                "A": {0: [0, 1], 1: [2, 3], 2: [4, 5], 3: [6]},
                "B": {0: [0], 1: [1, 2], 2: [3, 4], 3: [5]},
                "C": {1: [0, 1], 2: [2, 3], 3: [4]},
                "S": {1: [0, 1], 2: [2, 3], 3: [4]},
                "F": {1: [0], 2: [1, 2], 3: [3]},
                "G": {2: [0], 3: [1]},
                "H": {3: [0]},
            }
            SL = dict(A=-3, B=-2, C=-1, S=-1, F=0, G=2, H=3, I=5, J=6, K=7)
            NITER = 22

            # epilog: tile 15's I/J/K run in the same iterations as tile
            # 14's, in second-slot PE bands (doI2/doJ2/doK2)
            EPILOG = {
                "I": {19: [14, 15], 20: []},
                "J": {20: [14, 15], 21: []},
                "K": {21: [14, 15]},
            }

            def sched(ph, t):
                if ph in PROLOG and t <= 3:
                    return [i for i in PROLOG[ph].get(t, []) if i < NTILES]
                if ph in EPILOG and t in EPILOG[ph]:
                    return EPILOG[ph][t]
                i = t - SL[ph]
                if ph in PROLOG and t < 4:
                    return []
                return [i] if 0 <= i < NTILES else []

            def one(ph, t):
                lst = sched(ph, t)
                assert len(lst) <= 1
                return lst[0] if lst else None

            def doA(i):
                h1p = pmlp.tile([128, COLS], F32, tag="mlp", name="h1p")
                mm(h1p[:], w1, xg[i // 4][:, COLS * (i % 4) : COLS * (i % 4 + 1)])
                h1[i] = wp.tile([128, COLS], F16, tag="h1", name="h1")
                nc.vector.tensor_scalar(h1[i][:], h1p[:], b1, 0.0, ALU.add, ALU.max)

            def doB(i):
                h2p = pmlp.tile([64, COLS], F32, tag="mlp", name="h2p")
                mm(h2p[:], w2, h1[i][:])
                h2[i] = wp.tile([65, COLS], F16, tag="h2", name="h2")
                nc.vector.tensor_scalar(h2[i][0:64, :], h2p[:], b2, 0.0,
                                        ALU.add, ALU.max)

            def doC(i):
                prp = pmlp.tile([128, 32], F32, tag="mlp", name="prp")
                for c in range(4):
                    mm(prp[:, 8 * c : 8 * (c + 1)],
                       h2[i][0:65, 128 * c : 128 * (c + 1)], w3a)
                pre[i] = wp.tile([128, 32], F16, tag="pre", name="pre")
                nc.scalar.activation(pre[i][:], prp[:], AF.Tanh)

            def doS(i):
                csA[i] = wp.tile([128, 64], F16, tag="csA", name="csA")
                srcp = _ap(pre[i], 0, [[8, 4], [1, 8]])
                nc.scalar.activation(
                    _ap(csA[i], 0, [[16, 4], [1, 8]]), srcp, AF.Sin,
                    bias=pi2, scale=0.5)
                nc.scalar.activation(
                    _ap(csA[i], 8, [[16, 4], [1, 8]]), srcp, AF.Sin,
                    bias=zero, scale=-0.5)

            def doF(i):
                qp = wp.tile([128, 64], F16, tag="qp", name="qp")
                for a in range(2):
                    nc.gpsimd.tensor_mul(
                        _ap(qp, 2 * a, [[16, 4], [4, 4], [1, 2]]),
                        _ap(csA[i], 8 * a, [[16, 4], [2, 4], [0, 2]]),
                        _ap(csA[i], 1, [[16, 4], [2, 4], [8, 2]]),
                    )
                uv = wp.tile([128, 128], F16, tag="uv", name="uv")
                nc.gpsimd.tensor_mul(
                    _ap(uv, 0, [[16, 8], [4, 4], [1, 4]]),
                    _ap(qp, 0, [[8, 8], [1, 4], [0, 4]]),
                    _ap(qp, 4, [[8, 8], [0, 4], [1, 4]]),
                )
                sB[i] = wp.tile([128, 1024], F16, tag="sB", name="sB", bufs=4)
                nc.gpsimd.tensor_mul(
                    _ap(sB[i], 0, [[256, 2], [16, 16], [1, 16]]),
                    _ap(uv, 0, [[32, 2], [1, 16], [0, 16]]),
                    _ap(uv, 16, [[32, 2], [0, 16], [1, 16]]),
                )
                nc.gpsimd.tensor_mul(
                    _ap(sB[i], 512, [[256, 2], [16, 16], [1, 16]]),
                    _ap(uv, 64, [[32, 2], [1, 16], [0, 16]]),
                    _ap(uv, 80, [[32, 2], [0, 16], [1, 16]]),
                )

            # blocks (2,0),(3,0),(0,1)..(3,1) go through the PE (gmap cols
            # sequential -> sT cols 256:1024 contiguous); blocks (0,0),(1,0)
            # ride the otherwise-idle Sync XBAR-DMA path straight into sT
            _GBLK = [(2, 0), (3, 0), (0, 1), (1, 1), (2, 1), (3, 1)]

            def doGalloc(i):
                sT[i] = wp.tile([128, 1024], F16, tag="sT", name="sT", bufs=4)
                gmap[i] = pb.tile([128, 768], F16, tag="pb", name="gmap")
                for b in range(2):
                    nc.sync.dma_start_transpose(
                        sT[i][:, 128 * b : 128 * (b + 1)],
                        sB[i][:, 256 * b : 256 * b + 128])

            def doGtrans(i, k):
                b, h = _GBLK[k]
                nc.tensor.transpose(
                    gmap[i][:, 128 * k : 128 * (k + 1)],
                    sB[i][:, 256 * b + 128 * h : 256 * b + 128 * h + 128],
                    ident[:])

            def doGcopy(i):
                nc.vector.tensor_copy(sT[i][:, 256:1024], gmap[i][:])

            def doHalloc(i):
                yps[i] = [
                    py.tile([128, 1024], F32, tag="y", name="y01"),
                    py.tile([128, 1024], F32, tag="y", name="y23"),
                ]

            def hpair(i, c):
                yp = yps[i][c // 2][:, 512 * (c % 2) : 512 * (c % 2 + 1)]
                mm(yp, ct[:, 128 * c : 128 * (c + 1)], sT[i][:, 0:512],
                   start=True, stop=False)
                mm(yp, ct[:, 512 + 128 * c : 512 + 128 * (c + 1)],
                   sT[i][:, 512:1024], start=False, stop=True)

            def hsq(i, pair):
                dst = (sqa, sqb)[pair]
                dst[i] = wp.tile([128, 1024], F16, tag="sqa" if pair == 0 else "sqb",
                                 name="sq", bufs=4)
                nc.scalar.activation(dst[i][:], yps[i][pair][:], AF.Square)

            def doMlbAlloc():
                mlb_cur[0] = pb.tile([128, COLS], F32, tag="pb", name="mlb")

            def doI(i):
                h4p = mlb_cur[0][32:64, :]
                for c in range(4):
                    srcq = (sqa if c < 2 else sqb)[i][:, 512 * (c % 2) : 512 * (c % 2 + 1)]
                    mm(h4p, m4[:, 32 * c : 32 * (c + 1)], srcq,
                       start=(c == 0), stop=(c == 3))
                h4t = wp.tile([64, COLS], F16, tag="h4", name="h4")
                h4[i] = h4t
                nc.vector.tensor_scalar(h4t[32:64, :], h4p, b4, 0.0,
                                        ALU.add, ALU.max)

            def doJ(i):
                h5p = mlb_cur[0][64:80, :]
                mm(h5p, w5, h4[i][32:64, :])
                h5t = wp.tile([80, COLS], F16, tag="h5", name="h5")
                h5[i] = h5t
                nc.vector.tensor_scalar(h5t[64:80, :], h5p, b5, 0.0,
                                        ALU.add, ALU.max)

            def doK(i):
                kp = mlb_cur[0][0:1, :]
                mm(kp, w6, h5[i][64:80, :])
                nc.scalar.activation(out_all[0:1, COLS * i : COLS * (i + 1)], kp,
                                     AF.Copy)
                if i == 13:
                    # bulk output DMA early: only the last 2 tiles' outputs
                    # ride the epilog dependency chain
                    nc.scalar.dma_start(out_d[:, 0 : COLS * 14],
                                        out_all[0:1, 0 : COLS * 14])

            # second-slot variants: the epilog runs tile 15's I/J/K in the
            # same iterations as tile 14's, using disjoint PE bands / PSUM
            # rows (I out @96, J @96->32, K @32->64)
            def doI2(i):
                h4p = mlb_cur[0][96:128, :]
                for c in range(4):
                    srcq = (sqa if c < 2 else sqb)[i][:, 512 * (c % 2) : 512 * (c % 2 + 1)]
                    mm(h4p, m4[:, 32 * c : 32 * (c + 1)], srcq,
                       start=(c == 0), stop=(c == 3), tile_position=(0, 96))
                h4t = wp.tile([128, COLS], F16, tag="h4", name="h4b")
                h4[i] = h4t
                nc.vector.tensor_scalar(h4t[96:128, :], h4p, b4b, 0.0,
                                        ALU.add, ALU.max)

            def doJ2(i):
                h5p = mlb_cur[0][32:48, :]
                mm(h5p, w5b, h4[i][96:128, :], tile_position=(96, 32))
                h5t = wp.tile([48, COLS], F16, tag="h5", name="h5b")
                h5[i] = h5t
                nc.vector.tensor_scalar(h5t[32:48, :], h5p, b5b, 0.0,
                                        ALU.add, ALU.max)

            def doK2(i):
                kp = mlb_cur[0][64:65, :]
                mm(kp, w6b, h5[i][32:48, :])
                nc.scalar.activation(out_all[64:65, COLS * i : COLS * (i + 1)], kp,
                                     AF.Copy)
                nc.scalar.dma_start(out_d[:, COLS * 14 : COLS * 15],
                                    out_all[0:1, COLS * 14 : COLS * 15])
                nc.scalar.dma_start(out_d[:, COLS * i :],
                                    out_all[64:65, COLS * i :])

            for t in range(NITER):
                if t <= 3:
                    # prologue: sequential, PE has slack
                    for i in sched("A", t):
                        doA(i)
                    for i in sched("B", t):
                        doB(i)
                    for i in sched("C", t):
                        doC(i)
                    for i in sched("S", t):
                        doS(i)
                    for i in sched("F", t):
                        doF(i)
                    for i in sched("G", t):
                        doGalloc(i)
                        for k in range(6):
                            doGtrans(i, k)
                        doGcopy(i)
                    for i in sched("H", t):
                        doHalloc(i)
                        hpair(i, 0)
                        hpair(i, 1)
                        hsq(i, 0)
                        hpair(i, 2)
                        hpair(i, 3)
                        hsq(i, 1)
                    continue

                iA = one("A", t); iB = one("B", t); iC = one("C", t)
                iS = one("S", t); iF = one("F", t); iG = one("G", t)
                iH = one("H", t)
                iIs = sched("I", t); iJs = sched("J", t); iKs = sched("K", t)

                # Transposes + sT copy FIRST: their input (sB of the
                # front-running F phase) is long ready, and the early sT
                # copy frees the gmap bank for this iteration's mlb before
                # the I/J/K matmuls need it.
                if iIs or iJs or iKs:
                    doMlbAlloc()
                if iG is not None:
                    doGalloc(iG)
                    for k in range(6):
                        doGtrans(iG, k)
                    doGcopy(iG)
                for n, i in enumerate(iIs):
                    (doI if n == 0 else doI2)(i)
                # J/K/A: disjoint PE sub-tiles (row bands 1/2/0), issued
                # adjacently so they can run concurrently in the array
                for n, i in enumerate(iJs):
                    (doJ if n == 0 else doJ2)(i)
                for n, i in enumerate(iKs):
                    (doK if n == 0 else doK2)(i)
                if iA is not None:
                    doA(iA)
                if iH is not None:
                    doHalloc(iH)
                    hpair(iH, 0)
                    hpair(iH, 1)
                    hsq(iH, 0)
                if iB is not None:
                    doB(iB)
                if iH is not None:
                    hpair(iH, 2)
                    hpair(iH, 3)
                    hsq(iH, 1)
                # C/S/F feed iterations 1-3 ahead: slack-tolerant, so their
                # Scalar/GpSimd ops queue behind the critical squares
                if iC is not None:
                    doC(iC)
                if iS is not None:
                    doS(iS)
                if iF is not None:
                    doF(iF)
    nc.compile()
    return nc


_NC_CACHE = []

# test-harness hooks (unused in grading): set _TRACE to profile; the full
# BassKernelResults of the last run lands in _LAST_RESULTS[0].
_TRACE = False
_LAST_RESULTS = []


def _get_nc():
    if not _NC_CACHE:
        _NC_CACHE.append(_build_nc())
    return _NC_CACHE[0]


def kernel(**inputs):
    consts, b6 = _prep_consts(inputs)
    x = np.asarray(inputs["x"], np.float32)  # (65536, 16)
    xt_full = np.ascontiguousarray(x.T.astype(np.float16))  # (16, 65536)

    nc = _get_nc()
    in_maps = []
    for c in range(N_CORES):
        m = {"xt": np.ascontiguousarray(xt_full[:, c * B_CORE : (c + 1) * B_CORE])}
        m.update(consts)
        in_maps.append(m)
    res = run_bass_kernel_spmd(nc, in_maps, list(range(N_CORES)), trace=_TRACE)
    _LAST_RESULTS.clear()
    _LAST_RESULTS.append(res)
    out = np.concatenate([r["out"].reshape(B_CORE) for r in res.results])
    return (out.reshape(BATCH, 1) + b6).astype(np.float32)


# revision 31
# speedup vs baseline: 1.1526x; 1.1526x over previous
"""Trainium2 kernel for nn_EnhancedHybridModel (hybrid MLP + 8-qubit circuit).

Reformulation (exact, up to f32 rounding):
  * BatchNorms are inference-mode -> folded into the adjacent Linear.
  * The quantum circuit after RY-encoding uses shared weights, so it is one
    fixed complex matrix M (256x256).  The encoded state is a REAL product
    vector s[b] = kron_i [cos(pre_i/2), -sin(pre_i/2)].
  * q_out = |M s|^2 @ Z  ->  y = [Re M; Im M] @ s  (512x256 matmul),
    then q_out @ W4eff.T folds with the Z-projection into M4 (512x32):
    h4 = relu(y^2 @ M4 + b4eff).

Data parallel over 8 NeuronCores: batch 65536 -> 8192 rows/core.

v3 strategy (on top of the v2 batch-major pipeline):
  * 6 of 8 [128,128] state transposes run on the PE (transpose-mode writes
    into a PSUM tile laid out as sT[256:1024], one DVE copy evacuates the
    lot); 2 ride the otherwise-idle Sync XBAR-DMA path straight into sT.
    Issued at the START of each iteration (input sB is 2 iterations old)
    so the DMA has a full iteration of slack and the early sT copy frees
    the shared PSUM bank before this iteration's I/J/K matmuls claim it.
  * Small matmuls pack into disjoint PE sub-tiles via base-partition
    placement: W5 at SBUF rows 32:64 (row band 1, output at PSUM 64:80),
    W6 at rows 64:80 (band 2, output at PSUM 0:1).  With W1 in band 0 the
    A/J/K matmuls are issued adjacently and run concurrently in the array.
  * y^2 is squared in [128,1024] pair-ACTs (two PSUM banks per y-pair tile)
    halving the per-op overhead; PSUM stays within 8 banks by ring-sharing
    one pool between the mlb (h4p/h5p/kp) tile and the transpose target.
  * Engine balance: Scalar = tanh/sin/squares/out-copy, Vector = relu+bias
    evacuations + sT copy, GpSimd = the whole kron build chain.
  * PSUM ring reuse is always cross-iteration (phase order A,J,K,C,I,H,B)
    so no PE matmul waits on a same-iteration evacuation.
"""

import numpy as np

import concourse.bass as bass
import concourse.mybir as mybir
import concourse.tile as tile
from concourse import bacc
from concourse.masks import make_identity
from concourse.bass_utils import run_bass_kernel_spmd

F32 = mybir.dt.float32
F16 = mybir.dt.float16
AF = mybir.ActivationFunctionType
ALU = mybir.AluOpType

N_CORES = 8
BATCH = 65536
B_CORE = BATCH // N_CORES  # 8192
COLS = 512  # batch columns per tile (one PSUM bank of f32)
NTILES = B_CORE // COLS  # 16

N_QUBITS = 8
N_LAYERS = 3
DIM = 256
EPS = 1e-5

# ---------------------------------------------------------------- host math

_idx = np.arange(DIM)
_CNOT_PERMS = []
for _i in range(N_QUBITS):
    for _j in range(_i + 1, N_QUBITS):
        _c = (_idx >> (N_QUBITS - 1 - _i)) & 1
        _CNOT_PERMS.append(np.where(_c == 1, _idx ^ (1 << (N_QUBITS - 1 - _j)), _idx))
_Z_SIGNS = np.stack(
    [1.0 - 2.0 * ((_idx >> (N_QUBITS - 1 - i)) & 1) for i in range(N_QUBITS)], axis=1
).astype(np.float64)


def _rx(t):
    c, s = np.cos(t / 2), -1j * np.sin(t / 2)
    return np.array([[c, s], [s, c]], np.complex128)


def _ry(t):
    c, s = np.cos(t / 2), np.sin(t / 2)
    return np.array([[c, -s], [s, c]], np.complex128)


def _rz(t):
    e = np.exp(-0.5j * t)
    return np.array([[e, 0], [0, np.conj(e)]], np.complex128)


def _apply_gate(M, G, w):
    # reference einsum('st,bpsq->bptq', U, state): state'[t] = sum_s U[s,t] state[s]
    left = 2**w
    Mr = M.reshape(left, 2, -1, DIM)
    return np.einsum("st,psqj->ptqj", G, Mr).reshape(DIM, DIM)


def _build_circuit_matrix(q_weights):
    qw = np.asarray(q_weights, np.float64)
    M = np.eye(DIM, dtype=np.complex128)
    for l in range(N_LAYERS):
        for i in range(N_QUBITS):
            M = _apply_gate(M, _rx(qw[l, i, 0]), i)
            M = _apply_gate(M, _ry(qw[l, i, 1]), i)
            M = _apply_gate(M, _rz(qw[l, i, 2]), i)
        for perm in _CNOT_PERMS:
            M = M[perm, :]
    return M


def _fold_bn(W, b, g, bt, m, v):
    sc = np.asarray(g, np.float64) / np.sqrt(np.asarray(v, np.float64) + EPS)
    Weff = sc[:, None] * np.asarray(W, np.float64)
    beff = (np.asarray(b, np.float64) - np.asarray(m, np.float64)) * sc + np.asarray(
        bt, np.float64
    )
    return Weff, beff


WCOLS = 1369


def _prep_consts(inputs):
    f = {k: np.asarray(v, np.float64) for k, v in inputs.items() if k != "x"}
    W1e, b1e = _fold_bn(f["W1"], f["b1"], f["g1"], f["bt1"], f["m1"], f["v1"])
    W2e, b2e = _fold_bn(f["W2"], f["b2"], f["g2"], f["bt2"], f["m2"], f["v2"])
    W4e, b4e = _fold_bn(f["W4"], f["b4"], f["g4"], f["bt4"], f["m4"], f["v4"])
    M = _build_circuit_matrix(f["q_weights"])
    C = np.concatenate([M.real, M.imag], axis=0)  # (512, 256)
    Zst = np.concatenate([_Z_SIGNS, _Z_SIGNS], axis=0)  # (512, 8)
    M4 = Zst @ W4e.T  # (512, 32)

    bf = np.float16
    # WPACK fp16 [128, 1369]: ct | w2 | w1 | w3aug | w5@rows32 | w6@rows64 | m4
    wpk = np.zeros((128, WCOLS), bf)
    CT = np.ascontiguousarray(C.T).astype(bf)  # (256, 512)
    wpk[:, 0:512] = CT[0:128]
    wpk[:, 512:1024] = CT[128:256]
    wpk[0:128, 1024:1088] = np.ascontiguousarray(W2e.T).astype(bf)
    wpk[0:16, 1088:1216] = np.ascontiguousarray(W1e.T).astype(bf)
    wpk[0:64, 1216:1224] = np.ascontiguousarray(f["W3"].T).astype(bf)  # (64, 8)
    wpk[64:65, 1216:1224] = np.asarray(f["b3"]).reshape(1, 8).astype(bf)
    # W5 staged at partitions 32:64 so the J matmul runs in PE row band 1
    wpk[32:64, 1224:1240] = np.ascontiguousarray(f["W5"].T).astype(bf)
    # W6 staged at partitions 64:80 (row band 2)
    wpk[64:80, 1240:1241] = np.ascontiguousarray(f["W6"].T).astype(bf)
    # second-slot copies for the epilog-compressed last tile (tile 15 runs
    # its I/J/K in the same iterations as tile 14, in different PE bands)
    wpk[96:128, 1224:1240] = np.ascontiguousarray(f["W5"].T).astype(bf)
    wpk[32:48, 1240:1241] = np.ascontiguousarray(f["W6"].T).astype(bf)
    M4b = M4.astype(bf)  # (512, 32)
    for c in range(4):
        wpk[:, 1241 + 32 * c : 1241 + 32 * (c + 1)] = M4b[128 * c : 128 * (c + 1)]
    # BIASES f32 [128, 6]: b1 | b2 | b4@rows32 | b5@rows64 | pi/2 | 0
    bs = np.zeros((128, 6), np.float32)
    bs[0:128, 0] = b1e
    bs[0:64, 1] = b2e
    bs[32:64, 2] = b4e
    bs[96:128, 2] = b4e  # second slot (epilog tile 15)
    bs[64:80, 3] = f["b5"]
    bs[32:48, 3] = f["b5"]  # second slot (epilog tile 15)
    bs[0:128, 4] = np.pi / 2
    return {"WPACK": wpk, "BIASES": bs}, float(np.asarray(f["b6"]).reshape(()))


# ------------------------------------------------------------- bass program


def _ap(t, offset, dims):
    """Custom free-dim access pattern on a tile: keep its partition dim."""
    a = t[:]
    return bass.AP(a.tensor, a.offset + offset, [list(a.ap[0])] + [list(d) for d in dims])


def _build_nc():
    nc = bacc.Bacc("TRN2", target_bir_lowering=False, debug=False)

    xt = nc.dram_tensor("xt", [16, B_CORE], F16, kind="ExternalInput")
    wpk_d = nc.dram_tensor("WPACK", [128, WCOLS], F16, kind="ExternalInput")
    bs_d = nc.dram_tensor("BIASES", [128, 6], F32, kind="ExternalInput")
    out_d = nc.dram_tensor("out", [1, B_CORE], F32, kind="ExternalOutput")

    with tile.TileContext(nc) as tc:
        with (
            tc.tile_pool(name="const", bufs=1) as cp,
            tc.tile_pool(name="work", bufs=3) as wp,
            tc.tile_pool(name="pmlp", bufs=2, space="PSUM") as pmlp,
            tc.tile_pool(name="py", bufs=2, space="PSUM") as py,
            tc.tile_pool(name="pb", bufs=2, space="PSUM") as pb,
        ):
            ident = cp.tile([128, 128], F16)
            make_identity(nc, ident[:])
            wpk = cp.tile([128, WCOLS], F16)
            # small weights (w1/w2/w3/w5/w6/m4) first so the MLP phases can
            # start while the big circuit matrix streams in behind them; the
            # big ct block is issued LAST (after x/biases) since the H phase
            # only needs it several iterations in
            nc.scalar.dma_start(wpk[:, 1024:WCOLS], wpk_d[:, 1024:WCOLS])
            bs = cp.tile([128, 6], F32)
            tblpin = cp.tile([1, 1], F16, name="tblpin")
            # pin the ACT table set that Sin/Tanh/Square/Copy live in, so
            # the one table load happens here (overlapped with the DMAs)
            nc.scalar.activation(tblpin[:], bs[0:1, 0:1], AF.Silu)
            ct = wpk[:, 0:1024]
            w2 = wpk[:, 1024:1088]
            w1 = wpk[0:16, 1088:1216]
            w3a = wpk[0:65, 1216:1224]
            w5 = wpk[32:64, 1224:1240]
            w6 = wpk[64:80, 1240:1241]
            w5b = wpk[96:128, 1224:1240]
            w6b = wpk[32:48, 1240:1241]
            m4 = wpk[:, 1241:1369]
            b1 = bs[0:128, 0:1]
            b2 = bs[0:64, 1:2]
            b4 = bs[32:64, 2:3]
            b4b = bs[96:128, 2:3]
            b5 = bs[64:80, 3:4]
            b5b = bs[32:48, 3:4]
            pi2 = bs[0:128, 4:5]
            zero = bs[0:128, 5:6]
            xg = []
            for g in range(4):
                xg.append(cp.tile([16, 4 * COLS], F16, name=f"xg{g}", tag=f"xg{g}"))
                nc.sync.dma_start(xg[g][:], xt[:, 4 * COLS * g : 4 * COLS * (g + 1)])
                if g == 0:
                    nc.sync.dma_start(bs[:], bs_d[:])
            nc.scalar.dma_start(wpk[:, 0:1024], wpk_d[:, 0:1024])

            # row 0 collects tiles 0-14; row 64 collects the epilog-
            # compressed tile 15 (its kp lands at PSUM partition 64)
            out_all = cp.tile([65, B_CORE], F32)
            mm = nc.tensor.matmul

            # prewrite the ones-row into the h2 ring buffers (row 64 is the
            # bias row for the data-stationary W3 matmul; the per-tile act
            # only writes rows 0:64, so the ones persist across slot reuse)
            for _ in range(3):
                h2pre = wp.tile([65, COLS], F16, tag="h2", name="h2pre")
                nc.gpsimd.memset(h2pre[64:65, :], 1.0)

            h1 = [None] * NTILES
            h2 = [None] * NTILES
            pre = [None] * NTILES
            csA = [None] * NTILES
            sB = [None] * NTILES
            sT = [None] * NTILES
            gmap = [None] * NTILES
            sqa = [None] * NTILES
            sqb = [None] * NTILES
            h4 = [None] * NTILES
            h5 = [None] * NTILES
            yps = [None] * NTILES
            mlb_cur = [None]  # one shared [128, COLS] PSUM tile per iteration

            # compressed-prologue schedule: early phases run 2 tiles/iter
            # (PE is H-idle during pipe fill), steady state 1 tile/iter.
            PROLOG = {
                "A": {0: [0, 1], 1: [2, 3], 2: [4, 5], 3: [6]},
                "B": {0: [0], 1: [1, 2], 2: [3, 4], 3: [5]},
                "C": {1: [0, 1], 2: [2, 3], 3: [4]},
                "S": {1: [0, 1], 2: [2, 3], 3: [4]},
                "F": {1: [0], 2: [1, 2], 3: [3]},
                "G": {2: [0], 3: [1]},
                "H": {3: [0]},
            }
            SL = dict(A=-3, B=-2, C=-1, S=-1, F=0, G=2, H=3, I=5, J=6, K=7)
            NITER = 22

            # epilog: tile 15's I/J/K run in the same iterations as tile
            # 14's, in second-slot PE bands (doI2/doJ2/doK2)
            EPILOG = {
                "I": {19: [14, 15], 20: []},
                "J": {20: [14, 15], 21: []},
                "K": {21: [14, 15]},
            }

            def sched(ph, t):
                if ph in PROLOG and t <= 3:
                    return [i for i in PROLOG[ph].get(t, []) if i < NTILES]
                if ph in EPILOG and t in EPILOG[ph]:
                    return EPILOG[ph][t]
                i = t - SL[ph]
                if ph in PROLOG and t < 4:
                    return []
                return [i] if 0 <= i < NTILES else []

            def one(ph, t):
                lst = sched(ph, t)
                assert len(lst) <= 1
                return lst[0] if lst else None

            def doA(i):
                h1p = pmlp.tile([128, COLS], F32, tag="mlp", name="h1p")
                mm(h1p[:], w1, xg[i // 4][:, COLS * (i % 4) : COLS * (i % 4 + 1)])
                h1[i] = wp.tile([128, COLS], F16, tag="h1", name="h1")
                nc.vector.tensor_scalar(h1[i][:], h1p[:], b1, 0.0, ALU.add, ALU.max)

            def doB(i):
                h2p = pmlp.tile([64, COLS], F32, tag="mlp", name="h2p")
                mm(h2p[:], w2, h1[i][:])
                h2[i] = wp.tile([65, COLS], F16, tag="h2", name="h2")
                nc.vector.tensor_scalar(h2[i][0:64, :], h2p[:], b2, 0.0,
                                        ALU.add, ALU.max)

            def doC(i):
                prp = pmlp.tile([128, 32], F32, tag="mlp", name="prp")
                for c in range(4):
                    mm(prp[:, 8 * c : 8 * (c + 1)],
                       h2[i][0:65, 128 * c : 128 * (c + 1)], w3a)
                pre[i] = wp.tile([128, 32], F16, tag="pre", name="pre")
                nc.scalar.activation(pre[i][:], prp[:], AF.Tanh)

            def doS(i):
                csA[i] = wp.tile([128, 64], F16, tag="csA", name="csA")
                srcp = _ap(pre[i], 0, [[8, 4], [1, 8]])
                nc.scalar.activation(
                    _ap(csA[i], 0, [[16, 4], [1, 8]]), srcp, AF.Sin,
                    bias=pi2, scale=0.5)
                nc.scalar.activation(
                    _ap(csA[i], 8, [[16, 4], [1, 8]]), srcp, AF.Sin,
                    bias=zero, scale=-0.5)

            def doF(i):
                qp = wp.tile([128, 64], F16, tag="qp", name="qp")
                for a in range(2):
                    nc.gpsimd.tensor_mul(
                        _ap(qp, 2 * a, [[16, 4], [4, 4], [1, 2]]),
                        _ap(csA[i], 8 * a, [[16, 4], [2, 4], [0, 2]]),
                        _ap(csA[i], 1, [[16, 4], [2, 4], [8, 2]]),
                    )
                uv = wp.tile([128, 128], F16, tag="uv", name="uv")
                nc.gpsimd.tensor_mul(
                    _ap(uv, 0, [[16, 8], [4, 4], [1, 4]]),
                    _ap(qp, 0, [[8, 8], [1, 4], [0, 4]]),
                    _ap(qp, 4, [[8, 8], [0, 4], [1, 4]]),
                )
                sB[i] = wp.tile([128, 1024], F16, tag="sB", name="sB", bufs=4)
                nc.gpsimd.tensor_mul(
                    _ap(sB[i], 0, [[256, 2], [16, 16], [1, 16]]),
                    _ap(uv, 0, [[32, 2], [1, 16], [0, 16]]),
                    _ap(uv, 16, [[32, 2], [0, 16], [1, 16]]),
                )
                nc.gpsimd.tensor_mul(
                    _ap(sB[i], 512, [[256, 2], [16, 16], [1, 16]]),
                    _ap(uv, 64, [[32, 2], [1, 16], [0, 16]]),
                    _ap(uv, 80, [[32, 2], [0, 16], [1, 16]]),
                )

            # blocks (2,0),(3,0),(0,1)..(3,1) go through the PE (gmap cols
            # sequential -> sT cols 256:1024 contiguous); blocks (0,0),(1,0)
            # ride the otherwise-idle Sync XBAR-DMA path straight into sT
            _GBLK = [(2, 0), (3, 0), (0, 1), (1, 1), (2, 1), (3, 1)]

            def doGalloc(i):
                sT[i] = wp.tile([128, 1024], F16, tag="sT", name="sT", bufs=4)
                gmap[i] = pb.tile([128, 768], F16, tag="pb", name="gmap")
                for b in range(2):
                    nc.sync.dma_start_transpose(
                        sT[i][:, 128 * b : 128 * (b + 1)],
                        sB[i][:, 256 * b : 256 * b + 128])

            def doGtrans(i, k):
                b, h = _GBLK[k]
                nc.tensor.transpose(
                    gmap[i][:, 128 * k : 128 * (k + 1)],
                    sB[i][:, 256 * b + 128 * h : 256 * b + 128 * h + 128],
                    ident[:])

            def doGcopy(i):
                nc.vector.tensor_copy(sT[i][:, 256:1024], gmap[i][:])

            def doHalloc(i):
                yps[i] = [
                    py.tile([128, 1024], F32, tag="y", name="y01"),
                    py.tile([128, 1024], F32, tag="y", name="y23"),
                ]

            def hpair(i, c):
                yp = yps[i][c // 2][:, 512 * (c % 2) : 512 * (c % 2 + 1)]
                mm(yp, ct[:, 128 * c : 128 * (c + 1)], sT[i][:, 0:512],
                   start=True, stop=False)
                mm(yp, ct[:, 512 + 128 * c : 512 + 128 * (c + 1)],
                   sT[i][:, 512:1024], start=False, stop=True)

            def hsq(i, pair):
                dst = (sqa, sqb)[pair]
                dst[i] = wp.tile([128, 1024], F16, tag="sqa" if pair == 0 else "sqb",
                                 name="sq", bufs=4)
                nc.scalar.activation(dst[i][:], yps[i][pair][:], AF.Square)

            def doMlbAlloc():
                mlb_cur[0] = pb.tile([128, COLS], F32, tag="pb", name="mlb")

            def doI(i):
                h4p = mlb_cur[0][32:64, :]
                for c in range(4):
                    srcq = (sqa if c < 2 else sqb)[i][:, 512 * (c % 2) : 512 * (c % 2 + 1)]
                    mm(h4p, m4[:, 32 * c : 32 * (c + 1)], srcq,
                       start=(c == 0), stop=(c == 3))
                h4t = wp.tile([64, COLS], F16, tag="h4", name="h4")
                h4[i] = h4t
                nc.vector.tensor_scalar(h4t[32:64, :], h4p, b4, 0.0,
                                        ALU.add, ALU.max)

            def doJ(i):
                h5p = mlb_cur[0][64:80, :]
                mm(h5p, w5, h4[i][32:64, :])
                h5t = wp.tile([80, COLS], F16, tag="h5", name="h5")
                h5[i] = h5t
                nc.vector.tensor_scalar(h5t[64:80, :], h5p, b5, 0.0,
                                        ALU.add, ALU.max)

            def doK(i):
                kp = mlb_cur[0][0:1, :]
                mm(kp, w6, h5[i][64:80, :])
                nc.scalar.activation(out_all[0:1, COLS * i : COLS * (i + 1)], kp,
                                     AF.Copy)
                if i == 13:
                    # bulk output DMA early: only the last 2 tiles' outputs
                    # ride the epilog dependency chain
                    nc.scalar.dma_start(out_d[:, 0 : COLS * 14],
                                        out_all[0:1, 0 : COLS * 14])

            # second-slot variants: the epilog runs tile 15's I/J/K in the
            # same iterations as tile 14's, using disjoint PE bands / PSUM
            # rows (I out @96, J @96->32, K @32->64)
            def doI2(i):
                h4p = mlb_cur[0][96:128, :]
                for c in range(4):
                    srcq = (sqa if c < 2 else sqb)[i][:, 512 * (c % 2) : 512 * (c % 2 + 1)]
                    mm(h4p, m4[:, 32 * c : 32 * (c + 1)], srcq,
                       start=(c == 0), stop=(c == 3), tile_position=(0, 96))
                h4t = wp.tile([128, COLS], F16, tag="h4", name="h4b")
                h4[i] = h4t
                nc.vector.tensor_scalar(h4t[96:128, :], h4p, b4b, 0.0,
                                        ALU.add, ALU.max)

            def doJ2(i):
                h5p = mlb_cur[0][32:48, :]
                mm(h5p, w5b, h4[i][96:128, :], tile_position=(96, 32))
                h5t = wp.tile([48, COLS], F16, tag="h5", name="h5b")
                h5[i] = h5t
                nc.vector.tensor_scalar(h5t[32:48, :], h5p, b5b, 0.0,
                                        ALU.add, ALU.max)

            def doK2(i):
                kp = mlb_cur[0][64:65, :]
                mm(kp, w6b, h5[i][32:48, :])
                nc.scalar.activation(out_all[64:65, COLS * i : COLS * (i + 1)], kp,
                                     AF.Copy)
                nc.scalar.dma_start(out_d[:, COLS * 14 : COLS * 15],
                                    out_all[0:1, COLS * 14 : COLS * 15])
                nc.scalar.dma_start(out_d[:, COLS * i :],
                                    out_all[64:65, COLS * i :])

            def emit(ph, i):
                if ph == "A":
                    doA(i)
                elif ph == "B":
                    doB(i)
                elif ph == "C":
                    doC(i)
                elif ph == "S":
                    doS(i)
                elif ph == "F":
                    doF(i)
                elif ph == "G":
                    doGalloc(i)
                    for k in range(6):
                        doGtrans(i, k)
                    doGcopy(i)
                elif ph == "H":
                    doHalloc(i)
                    hpair(i, 0)
                    hpair(i, 1)
                    hsq(i, 0)
                    hpair(i, 2)
                    hpair(i, 3)
                    hsq(i, 1)

            # prologue emission: the leading tile's pipeline-critical ops
            # (C/S/F, then G, then H) are hoisted to the FRONT of each
            # iteration so they don't queue behind other tiles' blocked ops
            # in the strict per-engine FIFOs; all hoisted ops only consume
            # data emitted in earlier iterations.
            PRE = {
                0: [("A", 0), ("A", 1), ("B", 0)],
                1: [("C", 0), ("S", 0), ("F", 0), ("A", 2), ("A", 3),
                    ("B", 1), ("B", 2), ("C", 1), ("S", 1)],
                2: [("G", 0), ("A", 4), ("A", 5), ("B", 3), ("B", 4),
                    ("C", 2), ("C", 3), ("S", 2), ("S", 3), ("F", 1),
                    ("F", 2)],
                3: [("H", 0), ("G", 1), ("A", 6), ("B", 5), ("C", 4),
                    ("S", 4), ("F", 3)],
            }

            for t in range(NITER):
                if t <= 3:
                    for ph, i in PRE[t]:
                        emit(ph, i)
                    continue

                iA = one("A", t); iB = one("B", t); iC = one("C", t)
                iS = one("S", t); iF = one("F", t); iG = one("G", t)
                iH = one("H", t)
                iIs = sched("I", t); iJs = sched("J", t); iKs = sched("K", t)

                # Transposes + sT copy FIRST: their input (sB of the
                # front-running F phase) is long ready, and the early sT
                # copy frees the gmap bank for this iteration's mlb before
                # the I/J/K matmuls need it.
                if iIs or iJs or iKs:
                    doMlbAlloc()
                if iG is not None:
                    doGalloc(iG)
                    for k in range(6):
                        doGtrans(iG, k)
                    doGcopy(iG)
                for n, i in enumerate(iIs):
                    (doI if n == 0 else doI2)(i)
                # J/K/A: disjoint PE sub-tiles (row bands 1/2/0), issued
                # adjacently so they can run concurrently in the array
                for n, i in enumerate(iJs):
                    (doJ if n == 0 else doJ2)(i)
                for n, i in enumerate(iKs):
                    (doK if n == 0 else doK2)(i)
                if iA is not None:
                    doA(iA)
                if iH is not None:
                    doHalloc(iH)
                    hpair(iH, 0)
                    hpair(iH, 1)
                    hsq(iH, 0)
                if iB is not None:
                    doB(iB)
                if iH is not None:
                    hpair(iH, 2)
                    hpair(iH, 3)
                    hsq(iH, 1)
                # C/S/F feed iterations 1-3 ahead: slack-tolerant, so their
                # Scalar/GpSimd ops queue behind the critical squares
                if iC is not None:
                    doC(iC)
                if iS is not None:
                    doS(iS)
                if iF is not None:
                    doF(iF)
    nc.compile()
    return nc


_NC_CACHE = []

# test-harness hooks (unused in grading): set _TRACE to profile; the full
# BassKernelResults of the last run lands in _LAST_RESULTS[0].
_TRACE = False
_LAST_RESULTS = []


def _get_nc():
    if not _NC_CACHE:
        _NC_CACHE.append(_build_nc())
    return _NC_CACHE[0]


def kernel(**inputs):
    consts, b6 = _prep_consts(inputs)
    x = np.asarray(inputs["x"], np.float32)  # (65536, 16)
    xt_full = np.ascontiguousarray(x.T.astype(np.float16))  # (16, 65536)

    nc = _get_nc()
    in_maps = []
    for c in range(N_CORES):
        m = {"xt": np.ascontiguousarray(xt_full[:, c * B_CORE : (c + 1) * B_CORE])}
        m.update(consts)
        in_maps.append(m)
    res = run_bass_kernel_spmd(nc, in_maps, list(range(N_CORES)), trace=_TRACE)
    _LAST_RESULTS.clear()
    _LAST_RESULTS.append(res)
    out = np.concatenate([r["out"].reshape(B_CORE) for r in res.results])
    return (out.reshape(BATCH, 1) + b6).astype(np.float32)


# revision 32
# speedup vs baseline: 1.1929x; 1.0350x over previous
"""Trainium2 kernel for nn_EnhancedHybridModel (hybrid MLP + 8-qubit circuit).

Reformulation (exact, up to f32 rounding):
  * BatchNorms are inference-mode -> folded into the adjacent Linear.
  * The quantum circuit after RY-encoding uses shared weights, so it is one
    fixed complex matrix M (256x256).  The encoded state is a REAL product
    vector s[b] = kron_i [cos(pre_i/2), -sin(pre_i/2)].
  * q_out = |M s|^2 @ Z  ->  y = [Re M; Im M] @ s  (512x256 matmul),
    then q_out @ W4eff.T folds with the Z-projection into M4 (512x32):
    h4 = relu(y^2 @ M4 + b4eff).

Data parallel over 8 NeuronCores: batch 65536 -> 8192 rows/core.

v3 strategy (on top of the v2 batch-major pipeline):
  * 6 of 8 [128,128] state transposes run on the PE (transpose-mode writes
    into a PSUM tile laid out as sT[256:1024], one DVE copy evacuates the
    lot); 2 ride the otherwise-idle Sync XBAR-DMA path straight into sT.
    Issued at the START of each iteration (input sB is 2 iterations old)
    so the DMA has a full iteration of slack and the early sT copy frees
    the shared PSUM bank before this iteration's I/J/K matmuls claim it.
  * Small matmuls pack into disjoint PE sub-tiles via base-partition
    placement: W5 at SBUF rows 32:64 (row band 1, output at PSUM 64:80),
    W6 at rows 64:80 (band 2, output at PSUM 0:1).  With W1 in band 0 the
    A/J/K matmuls are issued adjacently and run concurrently in the array.
  * y^2 is squared in [128,1024] pair-ACTs (two PSUM banks per y-pair tile)
    halving the per-op overhead; PSUM stays within 8 banks by ring-sharing
    one pool between the mlb (h4p/h5p/kp) tile and the transpose target.
  * Engine balance: Scalar = tanh/sin/squares/out-copy, Vector = relu+bias
    evacuations + sT copy, GpSimd = the whole kron build chain.
  * PSUM ring reuse is always cross-iteration (phase order A,J,K,C,I,H,B)
    so no PE matmul waits on a same-iteration evacuation.
"""

import numpy as np

import concourse.bass as bass
import concourse.mybir as mybir
import concourse.tile as tile
from concourse import bacc
from concourse.masks import make_identity
from concourse.bass_utils import run_bass_kernel_spmd

F32 = mybir.dt.float32
F16 = mybir.dt.float16
AF = mybir.ActivationFunctionType
ALU = mybir.AluOpType

N_CORES = 8
BATCH = 65536
B_CORE = BATCH // N_CORES  # 8192
COLS = 512  # batch columns per tile (one PSUM bank of f32)
NTILES = B_CORE // COLS  # 16

N_QUBITS = 8
N_LAYERS = 3
DIM = 256
EPS = 1e-5

# ---------------------------------------------------------------- host math

_idx = np.arange(DIM)
_CNOT_PERMS = []
for _i in range(N_QUBITS):
    for _j in range(_i + 1, N_QUBITS):
        _c = (_idx >> (N_QUBITS - 1 - _i)) & 1
        _CNOT_PERMS.append(np.where(_c == 1, _idx ^ (1 << (N_QUBITS - 1 - _j)), _idx))
_Z_SIGNS = np.stack(
    [1.0 - 2.0 * ((_idx >> (N_QUBITS - 1 - i)) & 1) for i in range(N_QUBITS)], axis=1
).astype(np.float64)


def _rx(t):
    c, s = np.cos(t / 2), -1j * np.sin(t / 2)
    return np.array([[c, s], [s, c]], np.complex128)


def _ry(t):
    c, s = np.cos(t / 2), np.sin(t / 2)
    return np.array([[c, -s], [s, c]], np.complex128)


def _rz(t):
    e = np.exp(-0.5j * t)
    return np.array([[e, 0], [0, np.conj(e)]], np.complex128)


def _apply_gate(M, G, w):
    # reference einsum('st,bpsq->bptq', U, state): state'[t] = sum_s U[s,t] state[s]
    left = 2**w
    Mr = M.reshape(left, 2, -1, DIM)
    return np.einsum("st,psqj->ptqj", G, Mr).reshape(DIM, DIM)


def _build_circuit_matrix(q_weights):
    qw = np.asarray(q_weights, np.float64)
    M = np.eye(DIM, dtype=np.complex128)
    for l in range(N_LAYERS):
        for i in range(N_QUBITS):
            M = _apply_gate(M, _rx(qw[l, i, 0]), i)
            M = _apply_gate(M, _ry(qw[l, i, 1]), i)
            M = _apply_gate(M, _rz(qw[l, i, 2]), i)
        for perm in _CNOT_PERMS:
            M = M[perm, :]
    return M


def _fold_bn(W, b, g, bt, m, v):
    sc = np.asarray(g, np.float64) / np.sqrt(np.asarray(v, np.float64) + EPS)
    Weff = sc[:, None] * np.asarray(W, np.float64)
    beff = (np.asarray(b, np.float64) - np.asarray(m, np.float64)) * sc + np.asarray(
        bt, np.float64
    )
    return Weff, beff


WCOLS = 1369


def _prep_consts(inputs):
    f = {k: np.asarray(v, np.float64) for k, v in inputs.items() if k != "x"}
    W1e, b1e = _fold_bn(f["W1"], f["b1"], f["g1"], f["bt1"], f["m1"], f["v1"])
    W2e, b2e = _fold_bn(f["W2"], f["b2"], f["g2"], f["bt2"], f["m2"], f["v2"])
    W4e, b4e = _fold_bn(f["W4"], f["b4"], f["g4"], f["bt4"], f["m4"], f["v4"])
    M = _build_circuit_matrix(f["q_weights"])
    C = np.concatenate([M.real, M.imag], axis=0)  # (512, 256)
    Zst = np.concatenate([_Z_SIGNS, _Z_SIGNS], axis=0)  # (512, 8)
    M4 = Zst @ W4e.T  # (512, 32)

    bf = np.float16
    # WPACK fp16 [128, 1369]: ct | w2 | w1 | w3aug | w5@rows32 | w6@rows64 | m4
    wpk = np.zeros((128, WCOLS), bf)
    CT = np.ascontiguousarray(C.T).astype(bf)  # (256, 512)
    wpk[:, 0:512] = CT[0:128]
    wpk[:, 512:1024] = CT[128:256]
    wpk[0:128, 1024:1088] = np.ascontiguousarray(W2e.T).astype(bf)
    wpk[0:16, 1088:1216] = np.ascontiguousarray(W1e.T).astype(bf)
    wpk[0:64, 1216:1224] = np.ascontiguousarray(f["W3"].T).astype(bf)  # (64, 8)
    wpk[64:65, 1216:1224] = np.asarray(f["b3"]).reshape(1, 8).astype(bf)
    # W5 staged at partitions 32:64 so the J matmul runs in PE row band 1
    wpk[32:64, 1224:1240] = np.ascontiguousarray(f["W5"].T).astype(bf)
    # W6 staged at partitions 64:80 (row band 2)
    wpk[64:80, 1240:1241] = np.ascontiguousarray(f["W6"].T).astype(bf)
    # second-slot copies for the epilog-compressed last tile (tile 15 runs
    # its I/J/K in the same iterations as tile 14, in different PE bands)
    wpk[96:128, 1224:1240] = np.ascontiguousarray(f["W5"].T).astype(bf)
    wpk[32:48, 1240:1241] = np.ascontiguousarray(f["W6"].T).astype(bf)
    M4b = M4.astype(bf)  # (512, 32)
    for c in range(4):
        wpk[:, 1241 + 32 * c : 1241 + 32 * (c + 1)] = M4b[128 * c : 128 * (c + 1)]
    # BIASES f32 [128, 6]: b1 | b2 | b4@rows32 | b5@rows64 | pi/2 | 0
    bs = np.zeros((128, 6), np.float32)
    bs[0:128, 0] = b1e
    bs[0:64, 1] = b2e
    bs[32:64, 2] = b4e
    bs[96:128, 2] = b4e  # second slot (epilog tile 15)
    bs[64:80, 3] = f["b5"]
    bs[32:48, 3] = f["b5"]  # second slot (epilog tile 15)
    bs[0:128, 4] = np.pi / 2
    return {"WPACK": wpk, "BIASES": bs}, float(np.asarray(f["b6"]).reshape(()))


# ------------------------------------------------------------- bass program


def _ap(t, offset, dims):
    """Custom free-dim access pattern on a tile: keep its partition dim."""
    a = t[:]
    return bass.AP(a.tensor, a.offset + offset, [list(a.ap[0])] + [list(d) for d in dims])


def _build_nc():
    nc = bacc.Bacc("TRN2", target_bir_lowering=False, debug=False)

    xt = nc.dram_tensor("xt", [16, B_CORE], F16, kind="ExternalInput")
    wpk_d = nc.dram_tensor("WPACK", [128, WCOLS], F16, kind="ExternalInput")
    bs_d = nc.dram_tensor("BIASES", [128, 6], F32, kind="ExternalInput")
    out_d = nc.dram_tensor("out", [1, B_CORE], F32, kind="ExternalOutput")

    with tile.TileContext(nc) as tc:
        with (
            tc.tile_pool(name="const", bufs=1) as cp,
            tc.tile_pool(name="work", bufs=3) as wp,
            tc.tile_pool(name="pmlp", bufs=2, space="PSUM") as pmlp,
            tc.tile_pool(name="py", bufs=2, space="PSUM") as py,
            tc.tile_pool(name="pb", bufs=2, space="PSUM") as pb,
        ):
            ident = cp.tile([128, 128], F16)
            make_identity(nc, ident[:])
            wpk = cp.tile([128, WCOLS], F16)
            # small weights (w1/w2/w3/w5/w6/m4) first so the MLP phases can
            # start while the big circuit matrix streams in behind them; the
            # big ct block is issued LAST (after x/biases) since the H phase
            # only needs it several iterations in
            nc.scalar.dma_start(wpk[:, 1024:WCOLS], wpk_d[:, 1024:WCOLS])
            bs = cp.tile([128, 6], F32)
            tblpin = cp.tile([1, 1], F16, name="tblpin")
            # pin the ACT table set that Sin/Tanh/Square/Copy live in, so
            # the one table load happens here (overlapped with the DMAs)
            nc.scalar.activation(tblpin[:], bs[0:1, 0:1], AF.Silu)
            ct = wpk[:, 0:1024]
            w2 = wpk[:, 1024:1088]
            w1 = wpk[0:16, 1088:1216]
            w3a = wpk[0:65, 1216:1224]
            w5 = wpk[32:64, 1224:1240]
            w6 = wpk[64:80, 1240:1241]
            w5b = wpk[96:128, 1224:1240]
            w6b = wpk[32:48, 1240:1241]
            m4 = wpk[:, 1241:1369]
            b1 = bs[0:128, 0:1]
            b2 = bs[0:64, 1:2]
            b4 = bs[32:64, 2:3]
            b4b = bs[96:128, 2:3]
            b5 = bs[64:80, 3:4]
            b5b = bs[32:48, 3:4]
            pi2 = bs[0:128, 4:5]
            zero = bs[0:128, 5:6]
            xg = []
            for g in range(4):
                xg.append(cp.tile([16, 4 * COLS], F16, name=f"xg{g}", tag=f"xg{g}"))
                nc.sync.dma_start(xg[g][:], xt[:, 4 * COLS * g : 4 * COLS * (g + 1)])
                if g == 0:
                    nc.sync.dma_start(bs[:], bs_d[:])
            nc.scalar.dma_start(wpk[:, 0:1024], wpk_d[:, 0:1024])

            # row 0 collects tiles 0-14; row 64 collects the epilog-
            # compressed tile 15 (its kp lands at PSUM partition 64)
            out_all = cp.tile([65, B_CORE], F32)
            mm = nc.tensor.matmul

            # prewrite the ones-row into the h2 ring buffers (row 64 is the
            # bias row for the data-stationary W3 matmul; the per-tile act
            # only writes rows 0:64, so the ones persist across slot reuse)
            for _ in range(3):
                h2pre = wp.tile([65, COLS], F16, tag="h2", name="h2pre")
                nc.gpsimd.memset(h2pre[64:65, :], 1.0)

            h1 = [None] * NTILES
            h2 = [None] * NTILES
            pre = [None] * NTILES
            csA = [None] * NTILES
            sB = [None] * NTILES
            sT = [None] * NTILES
            gmap = [None] * NTILES
            sqa = [None] * NTILES
            sqb = [None] * NTILES
            h4 = [None] * NTILES
            h5 = [None] * NTILES
            yps = [None] * NTILES
            mlb_cur = [None]  # one shared [128, COLS] PSUM tile per iteration

            # compressed-prologue schedule: early phases run 2 tiles/iter
            # (PE is H-idle during pipe fill), steady state 1 tile/iter.
            PROLOG = {
                "A": {0: [0, 1], 1: [2, 3], 2: [4, 5], 3: [6]},
                "B": {0: [0], 1: [1, 2], 2: [3, 4], 3: [5]},
                "C": {1: [0, 1], 2: [2, 3], 3: [4]},
                "S": {1: [0, 1], 2: [2, 3], 3: [4]},
                "F": {1: [0], 2: [1, 2], 3: [3]},
                "G": {2: [0], 3: [1]},
                "H": {3: [0]},
            }
            SL = dict(A=-3, B=-2, C=-1, S=-1, F=0, G=2, H=3, I=5, J=6, K=7)
            NITER = 22

            # epilog: tile 15's I/J/K run in the same iterations as tile
            # 14's, in second-slot PE bands (doI2/doJ2/doK2)
            EPILOG = {
                "I": {19: [14, 15], 20: []},
                "J": {20: [14, 15], 21: []},
                "K": {21: [14, 15]},
            }

            def sched(ph, t):
                if ph in PROLOG and t <= 3:
                    return [i for i in PROLOG[ph].get(t, []) if i < NTILES]
                if ph in EPILOG and t in EPILOG[ph]:
                    return EPILOG[ph][t]
                i = t - SL[ph]
                if ph in PROLOG and t < 4:
                    return []
                return [i] if 0 <= i < NTILES else []

            def one(ph, t):
                lst = sched(ph, t)
                assert len(lst) <= 1
                return lst[0] if lst else None

            def doA(i):
                h1p = pmlp.tile([128, COLS], F32, tag="mlp", name="h1p")
                mm(h1p[:], w1, xg[i // 4][:, COLS * (i % 4) : COLS * (i % 4 + 1)])
                h1[i] = wp.tile([128, COLS], F16, tag="h1", name="h1")
                nc.vector.tensor_scalar(h1[i][:], h1p[:], b1, 0.0, ALU.add, ALU.max)

            def doB(i):
                h2p = pmlp.tile([64, COLS], F32, tag="mlp", name="h2p")
                mm(h2p[:], w2, h1[i][:])
                h2[i] = wp.tile([65, COLS], F16, tag="h2", name="h2")
                nc.vector.tensor_scalar(h2[i][0:64, :], h2p[:], b2, 0.0,
                                        ALU.add, ALU.max)

            def doC(i):
                prp = pmlp.tile([128, 32], F32, tag="mlp", name="prp")
                for c in range(4):
                    mm(prp[:, 8 * c : 8 * (c + 1)],
                       h2[i][0:65, 128 * c : 128 * (c + 1)], w3a)
                pre[i] = wp.tile([128, 32], F16, tag="pre", name="pre")
                nc.scalar.activation(pre[i][:], prp[:], AF.Tanh)

            def doS(i):
                csA[i] = wp.tile([128, 64], F16, tag="csA", name="csA")
                srcp = _ap(pre[i], 0, [[8, 4], [1, 8]])
                nc.scalar.activation(
                    _ap(csA[i], 0, [[16, 4], [1, 8]]), srcp, AF.Sin,
                    bias=pi2, scale=0.5)
                nc.scalar.activation(
                    _ap(csA[i], 8, [[16, 4], [1, 8]]), srcp, AF.Sin,
                    bias=zero, scale=-0.5)

            def doF(i):
                qp = wp.tile([128, 64], F16, tag="qp", name="qp")
                for a in range(2):
                    nc.gpsimd.tensor_mul(
                        _ap(qp, 2 * a, [[16, 4], [4, 4], [1, 2]]),
                        _ap(csA[i], 8 * a, [[16, 4], [2, 4], [0, 2]]),
                        _ap(csA[i], 1, [[16, 4], [2, 4], [8, 2]]),
                    )
                uv = wp.tile([128, 128], F16, tag="uv", name="uv")
                nc.gpsimd.tensor_mul(
                    _ap(uv, 0, [[16, 8], [4, 4], [1, 4]]),
                    _ap(qp, 0, [[8, 8], [1, 4], [0, 4]]),
                    _ap(qp, 4, [[8, 8], [0, 4], [1, 4]]),
                )
                sB[i] = wp.tile([128, 1024], F16, tag="sB", name="sB", bufs=4)
                nc.gpsimd.tensor_mul(
                    _ap(sB[i], 0, [[256, 2], [16, 16], [1, 16]]),
                    _ap(uv, 0, [[32, 2], [1, 16], [0, 16]]),
                    _ap(uv, 16, [[32, 2], [0, 16], [1, 16]]),
                )
                nc.gpsimd.tensor_mul(
                    _ap(sB[i], 512, [[256, 2], [16, 16], [1, 16]]),
                    _ap(uv, 64, [[32, 2], [1, 16], [0, 16]]),
                    _ap(uv, 80, [[32, 2], [0, 16], [1, 16]]),
                )

            # blocks (2,0),(3,0),(0,1)..(3,1) go through the PE (gmap cols
            # sequential -> sT cols 256:1024 contiguous); blocks (0,0),(1,0)
            # ride the otherwise-idle Sync XBAR-DMA path straight into sT
            _GBLK = [(2, 0), (3, 0), (0, 1), (1, 1), (2, 1), (3, 1)]

            def doGalloc(i):
                sT[i] = wp.tile([128, 1024], F16, tag="sT", name="sT", bufs=4)
                gmap[i] = pb.tile([128, 768], F16, tag="pb", name="gmap")
                for b in range(2):
                    nc.sync.dma_start_transpose(
                        sT[i][:, 128 * b : 128 * (b + 1)],
                        sB[i][:, 256 * b : 256 * b + 128])

            def doGtrans(i, k):
                b, h = _GBLK[k]
                nc.tensor.transpose(
                    gmap[i][:, 128 * k : 128 * (k + 1)],
                    sB[i][:, 256 * b + 128 * h : 256 * b + 128 * h + 128],
                    ident[:])

            def doGcopy(i):
                nc.vector.tensor_copy(sT[i][:, 256:1024], gmap[i][:])

            def doHalloc(i):
                yps[i] = [
                    py.tile([128, 1024], F32, tag="y", name="y01"),
                    py.tile([128, 1024], F32, tag="y", name="y23"),
                ]

            def hpair(i, c):
                yp = yps[i][c // 2][:, 512 * (c % 2) : 512 * (c % 2 + 1)]
                mm(yp, ct[:, 128 * c : 128 * (c + 1)], sT[i][:, 0:512],
                   start=True, stop=False)
                mm(yp, ct[:, 512 + 128 * c : 512 + 128 * (c + 1)],
                   sT[i][:, 512:1024], start=False, stop=True)

            def hsq(i, pair):
                dst = (sqa, sqb)[pair]
                dst[i] = wp.tile([128, 1024], F16, tag="sqa" if pair == 0 else "sqb",
                                 name="sq", bufs=4)
                nc.scalar.activation(dst[i][:], yps[i][pair][:], AF.Square)

            def doMlbAlloc():
                mlb_cur[0] = pb.tile([128, COLS], F32, tag="pb", name="mlb")

            def doI(i):
                h4p = mlb_cur[0][32:64, :]
                for c in range(4):
                    srcq = (sqa if c < 2 else sqb)[i][:, 512 * (c % 2) : 512 * (c % 2 + 1)]
                    mm(h4p, m4[:, 32 * c : 32 * (c + 1)], srcq,
                       start=(c == 0), stop=(c == 3))
                h4t = wp.tile([64, COLS], F16, tag="h4", name="h4")
                h4[i] = h4t
                nc.vector.tensor_scalar(h4t[32:64, :], h4p, b4, 0.0,
                                        ALU.add, ALU.max)

            def doJ(i):
                h5p = mlb_cur[0][64:80, :]
                mm(h5p, w5, h4[i][32:64, :])
                h5t = wp.tile([80, COLS], F16, tag="h5", name="h5")
                h5[i] = h5t
                nc.vector.tensor_scalar(h5t[64:80, :], h5p, b5, 0.0,
                                        ALU.add, ALU.max)

            def doK(i):
                kp = mlb_cur[0][0:1, :]
                mm(kp, w6, h5[i][64:80, :])
                nc.scalar.activation(out_all[0:1, COLS * i : COLS * (i + 1)], kp,
                                     AF.Copy)
                if i == 13:
                    # bulk output DMA early: only the last 2 tiles' outputs
                    # ride the epilog dependency chain
                    nc.scalar.dma_start(out_d[:, 0 : COLS * 14],
                                        out_all[0:1, 0 : COLS * 14])

            # second-slot variants: the epilog runs tile 15's I/J/K in the
            # same iterations as tile 14's, using disjoint PE bands / PSUM
            # rows (I out @96, J @96->32, K @32->64)
            def doI2(i):
                h4p = mlb_cur[0][96:128, :]
                for c in range(4):
                    srcq = (sqa if c < 2 else sqb)[i][:, 512 * (c % 2) : 512 * (c % 2 + 1)]
                    mm(h4p, m4[:, 32 * c : 32 * (c + 1)], srcq,
                       start=(c == 0), stop=(c == 3), tile_position=(0, 96))
                h4t = wp.tile([128, COLS], F16, tag="h4", name="h4b")
                h4[i] = h4t
                nc.vector.tensor_scalar(h4t[96:128, :], h4p, b4b, 0.0,
                                        ALU.add, ALU.max)

            def doJ2(i):
                h5p = mlb_cur[0][32:48, :]
                mm(h5p, w5b, h4[i][96:128, :], tile_position=(96, 32))
                h5t = wp.tile([48, COLS], F16, tag="h5", name="h5b")
                h5[i] = h5t
                nc.vector.tensor_scalar(h5t[32:48, :], h5p, b5b, 0.0,
                                        ALU.add, ALU.max)

            def doK2(i):
                kp = mlb_cur[0][64:65, :]
                mm(kp, w6b, h5[i][32:48, :])
                nc.scalar.activation(out_all[64:65, COLS * i : COLS * (i + 1)], kp,
                                     AF.Copy)
                nc.scalar.dma_start(out_d[:, COLS * 14 : COLS * 15],
                                    out_all[0:1, COLS * 14 : COLS * 15])
                nc.scalar.dma_start(out_d[:, COLS * i :],
                                    out_all[64:65, COLS * i :])

            for t in range(NITER):
                if t <= 3:
                    # prologue: sequential, PE has slack
                    for i in sched("A", t):
                        doA(i)
                    for i in sched("B", t):
                        doB(i)
                    for i in sched("C", t):
                        doC(i)
                    for i in sched("S", t):
                        doS(i)
                    for i in sched("F", t):
                        doF(i)
                    for i in sched("G", t):
                        doGalloc(i)
                        for k in range(6):
                            doGtrans(i, k)
                        doGcopy(i)
                    for i in sched("H", t):
                        doHalloc(i)
                        hpair(i, 0)
                        hpair(i, 1)
                        hsq(i, 0)
                        hpair(i, 2)
                        hpair(i, 3)
                        hsq(i, 1)
                    continue

                iA = one("A", t); iB = one("B", t); iC = one("C", t)
                iS = one("S", t); iF = one("F", t); iG = one("G", t)
                iH = one("H", t)
                iIs = sched("I", t); iJs = sched("J", t); iKs = sched("K", t)

                # Transposes + sT copy FIRST: their input (sB of the
                # front-running F phase) is long ready, and the early sT
                # copy frees the gmap bank for this iteration's mlb before
                # the I/J/K matmuls need it.
                if iIs or iJs or iKs:
                    doMlbAlloc()
                if iG is not None:
                    doGalloc(iG)
                    for k in range(6):
                        doGtrans(iG, k)
                    doGcopy(iG)
                for n, i in enumerate(iIs):
                    (doI if n == 0 else doI2)(i)
                # J/K/A: disjoint PE sub-tiles (row bands 1/2/0), issued
                # adjacently so they can run concurrently in the array
                for n, i in enumerate(iJs):
                    (doJ if n == 0 else doJ2)(i)
                for n, i in enumerate(iKs):
                    (doK if n == 0 else doK2)(i)
                if iA is not None:
                    doA(iA)
                if iH is not None:
                    doHalloc(iH)
                    hpair(iH, 0)
                    hpair(iH, 1)
                    hsq(iH, 0)
                if iB is not None:
                    doB(iB)
                if iH is not None:
                    hpair(iH, 2)
                    hpair(iH, 3)
                    hsq(iH, 1)
                # C/S/F feed iterations 1-3 ahead: slack-tolerant, so their
                # Scalar/GpSimd ops queue behind the critical squares
                if iC is not None:
                    doC(iC)
                if iS is not None:
                    doS(iS)
                if iF is not None:
                    doF(iF)
    nc.compile()
    return nc


_NC_CACHE = []

# test-harness hooks (unused in grading): set _TRACE to profile; the full
# BassKernelResults of the last run lands in _LAST_RESULTS[0].
_TRACE = False
_LAST_RESULTS = []


def _get_nc():
    if not _NC_CACHE:
        _NC_CACHE.append(_build_nc())
    return _NC_CACHE[0]


def kernel(**inputs):
    consts, b6 = _prep_consts(inputs)
    x = np.asarray(inputs["x"], np.float32)  # (65536, 16)
    xt_full = np.ascontiguousarray(x.T.astype(np.float16))  # (16, 65536)

    nc = _get_nc()
    in_maps = []
    for c in range(N_CORES):
        m = {"xt": np.ascontiguousarray(xt_full[:, c * B_CORE : (c + 1) * B_CORE])}
        m.update(consts)
        in_maps.append(m)
    res = run_bass_kernel_spmd(nc, in_maps, list(range(N_CORES)), trace=_TRACE)
    _LAST_RESULTS.clear()
    _LAST_RESULTS.append(res)
    out = np.concatenate([r["out"].reshape(B_CORE) for r in res.results])
    return (out.reshape(BATCH, 1) + b6).astype(np.float32)
